# revision 20
# baseline (speedup 1.0000x reference)
"""ChildSum TreeLSTM on 8 trn2 NeuronCores (Bass/Tile, SPMD feature-split).

Strategy
--------
head[j] > j, so the tree is topologically ordered. Nodes are relabeled
level-contiguously (leaves first). Hidden dim H=1024 is feature-split
across 8 cores (128 features each). Per level (processed in batches of
<=512 nodes):

  gates_p = sigmoid/tanh(Wx_p + sum_{k in ch(p)} (U g h_k))

Linearity: g_k = [U_i h_k; U_o h_k; U_u h_k; U_f h_k] (each core computes
its 4x128 slice) is computed once at k's own level (batched matmul, large
N), stored node-major in DRAM; parents segment-sum gathered g rows with a
one-hot S matmul on the PE. The forget path is nonlinear per child:
fc_p = sum_k sigmoid(Wxf_p + (U_f h_k)) * c_k, handled with gathered
rows + elementwise + the same S matmul. Only h needs cross-core comm:
one AllGather per batch (h slice [128,m] -> full h^T [1024,m] feat-major,
which feeds the g matmul directly).
"""
import numpy as np

N = 4096
H = 1024
HC = 128
NCORES = 8
PAD = N            # pad row index in node-major stores
BATCH = 512
CH = 128           # children per chunk
KCH = H // 128     # contraction chunks for U matmuls
KCHX = KCH + 1     # x contraction chunks incl. bias row
MAXNCH = 8


def _wrap_idx(a):
    """dma_gather index layout: idx[i] at [i%16, i//16], tiled to 128 partitions."""
    a = np.asarray(a, np.int64)
    n = len(a)
    c = (n + 15) // 16
    w = np.zeros((16, c), np.int16)
    w[np.arange(n) % 16, np.arange(n) // 16] = a.astype(np.int16)
    return np.tile(w, (8, 1))


def _schedule(head):
    head = np.asarray(head).astype(np.int64)
    n = head.shape[0]
    lev = np.zeros(n + 1, np.int64)
    for k in range(n):
        p = head[k]
        if lev[p] < lev[k] + 1:
            lev[p] = lev[k] + 1
    lv = lev[:n]
    order = np.argsort(lv, kind="stable")          # new -> old
    new_of_old = np.empty(n, np.int64)
    new_of_old[order] = np.arange(n)
    head_new = np.full(n, n, np.int64)
    for old in range(n):
        p = head[old]
        head_new[new_of_old[old]] = new_of_old[p] if p < n else n
    nlev = int(lv.max()) + 1
    mlev = [int((lv == L).sum()) for L in range(nlev)]
    start = np.concatenate([[0], np.cumsum(mlev)])
    kids = [[] for _ in range(n)]
    for k in range(n):
        p = head_new[k]
        if p < n:
            kids[p].append(k)

    batches = []
    for L in range(nlev):
        gs = int(start[L])
        while gs < start[L + 1]:
            bm = int(min(BATCH, start[L + 1] - gs))
            batches.append([L, gs, bm])
            gs += bm

    idx_blocks = []      # int16 wrapped blocks, concat on axis 1
    s_blocks = []        # [128, win] fp32 blocks
    icol = 0
    scol = 0
    binfos = []
    for (L, gs, bm) in batches:
        if L == 0:
            binfos.append(dict(L=L, gs=gs, bm=bm, chunks=[], nch=0))
            continue
        chunks = []      # (wlo_rel, win, s_off_rel)
        slots_all = []
        wxf_all = []
        cur, curp = [], []
        plo = [None]
        phi = [None]

        def emit():
            padn = CH - len(cur)
            slots_all.extend(cur + [PAD] * padn)
            wxf_all.extend(curp + [PAD] * padn)
            win = phi[0] - plo[0] + 1
            S = np.zeros((CH, win), np.float32)
            for s in range(len(curp)):
                S[s, curp[s] - plo[0]] = 1.0
            chunks.append((plo[0] - gs, win))
            s_blocks.append(S)
            cur.clear()
            curp.clear()
            plo[0] = None

        for p in range(gs, gs + bm):
            ck = kids[p]
            assert 1 <= len(ck) <= CH
            if cur and len(cur) + len(ck) > CH:
                emit()
            if plo[0] is None:
                plo[0] = p
            phi[0] = p
            cur.extend(ck)
            curp.extend([p] * len(ck))
        if cur:
            emit()
        nch = len(chunks)
        assert nch <= MAXNCH, nch
        wi = _wrap_idx(slots_all)
        ww = _wrap_idx(wxf_all)
        # per-chunk S col offsets (relative to this batch's scol)
        ch2 = []
        so = 0
        for (wlo, win) in chunks:
            ch2.append((wlo, win, so))
            so += win
        binfos.append(dict(L=L, gs=gs, bm=bm, chunks=ch2, nch=nch,
                           icol_child=icol, icol_wxf=icol + wi.shape[1],
                           scol=scol, scols=so))
        idx_blocks.append(wi)
        idx_blocks.append(ww)
        icol += wi.shape[1] + ww.shape[1]
        scol += so

    idxt = (np.concatenate(idx_blocks, axis=1) if idx_blocks
            else np.zeros((128, 1), np.int16))
    sall = (np.concatenate(s_blocks, axis=1) if s_blocks
            else np.zeros((128, 1), np.float32))
    return dict(order=order, new_of_old=new_of_old, nlev=nlev,
                batches=binfos, idxt=idxt, sall=sall)


def _build_nc(sched, mode="full"):
    import concourse.mybir as mybir
    import concourse.tile as tile
    from concourse import bacc
    from concourse.masks import make_identity

    F32 = mybir.dt.float32
    F32R = mybir.dt.float32r
    I16 = mybir.dt.int16
    SIG = mybir.ActivationFunctionType.Sigmoid
    TANH = mybir.ActivationFunctionType.Tanh

    binfos = sched["batches"]
    nlev = sched["nlev"]
    icols = sched["idxt"].shape[1]
    scols = sched["sall"].shape[1]

    nc = bacc.Bacc("TRN2", target_bir_lowering=False, debug=False,
                   num_devices=NCORES)
    xT = nc.declare_dram_parameter("xT", [KCHX * 128, N], F32R, isOutput=False)
    WT = nc.declare_dram_parameter("WT", [KCHX * 128, 512], F32R, isOutput=False)
    UT = nc.declare_dram_parameter("UT", [H, 512], F32R, isOutput=False)
    SALL = nc.declare_dram_parameter("SALL", [128, scols], F32, isOutput=False)
    IDXT = nc.declare_dram_parameter("IDXT", [128, icols], I16, isOutput=False)
    h_out = nc.declare_dram_parameter("h_out", [HC, N], F32, isOutput=True)
    c_out = nc.declare_dram_parameter("c_out", [N + 1, HC], F32, isOutput=True)

    g_store = nc.dram_tensor("g_store", [N + 1, 512], F32)
    wxf_store = nc.dram_tensor("wxf_store", [N + 1, HC], F32)
    wx_dram = nc.dram_tensor("wx_dram", [128, 3 * N], F32)  # i,o,u feat-major
    ag_ins, ag_outs = [], []
    for bi, b in enumerate(binfos):
        last = (b["L"] == nlev - 1)
        if last:
            ag_ins.append(None)
            ag_outs.append(None)
        else:
            ag_ins.append(nc.dram_tensor(f"agi{bi}", [128, b["bm"]], F32R))
            ag_outs.append(nc.dram_tensor(f"ago{bi}", [H, b["bm"]], F32R,
                                          addr_space="Shared"))

    ecnt = [0]

    def cpcopy(out, in_):
        ecnt[0] += 1
        if ecnt[0] % 2:
            nc.vector.tensor_copy(out, in_)
        else:
            nc.scalar.copy(out, in_)

    with tile.TileContext(nc) as tc:
        with (
            tc.tile_pool(name="const", bufs=1) as cpool,
            tc.tile_pool(name="xt", bufs=3) as xtp,
            tc.tile_pool(name="work", bufs=2) as wp,
            tc.tile_pool(name="gt", bufs=1) as gtp,
            tc.tile_pool(name="psA", bufs=1, space="PSUM") as psA,
            tc.tile_pool(name="pst", bufs=2, space="PSUM") as pst,
        ):
            ident = cpool.tile([128, 128], F32)
            make_identity(nc, ident[:])
            wt_sb = cpool.tile([128, KCHX, 512], F32R)
            nc.sync.dma_start(wt_sb[:], WT[:].rearrange("(k p) j -> p k j", p=128))
            ut_sb = cpool.tile([128, KCH, 512], F32R)
            nc.sync.dma_start(ut_sb[:], UT[:].rearrange("(k p) j -> p k j", p=128))
            idx_sb = cpool.tile([128, icols], I16)
            nc.sync.dma_start(idx_sb[:], IDXT[:])
            sall_sb = cpool.tile([128, scols], F32)
            nc.sync.dma_start(sall_sb[:], SALL[:])
            zrow = cpool.tile([1, 512], F32)
            nc.vector.memset(zrow[:], 0.0)
            nc.sync.dma_start(g_store[N:N + 1, :], zrow[:, :])
            nc.sync.dma_start(wxf_store[N:N + 1, :], zrow[:, :HC])

            # ---------------- Wx phase ----------------
            # order: chunk 0 (leaves first), then the chunks containing all
            # parents (wxf consumers), then the rest.
            nchunks = N // 512
            lev1 = binfos[0]["bm"]  # not reliable; compute from sched
            # first chunk containing a level>=1 node:
            l1start = None
            for b in binfos:
                if b["L"] == 1:
                    l1start = b["gs"]
                    break
            if l1start is None:
                l1start = N
            pstart = l1start // 512
            order_chunks = ([0] + list(range(pstart, nchunks)) +
                            [c for c in range(1, pstart)])
            for ci in order_chunks:
                ps_wx = [psA.tile([128, 512], F32, tag=f"A{g}", name=f"pswx{g}") for g in range(4)]
                for k in range(KCHX):
                    xt_t = xtp.tile([128, 512], F32R, tag="xt")
                    nc.sync.dma_start(
                        xt_t[:], xT[k * 128:(k + 1) * 128, ci * 512:(ci + 1) * 512])
                    for g in range(4):
                        nc.tensor.matmul(
                            ps_wx[g][:], wt_sb[:, k, g * 128:(g + 1) * 128],
                            xt_t[:], start=(k == 0), stop=(k == KCHX - 1))
                for g in range(3):
                    t = wp.tile([128, 512], F32, tag="wxcp")
                    cpcopy(t[:], ps_wx[g][:])
                    nc.sync.dma_start(
                        wx_dram[:, g * N + ci * 512: g * N + (ci + 1) * 512], t[:])
                # f gate: transpose to node-major wxf_store
                tf = wp.tile([128, 512], F32, tag="wxf")
                cpcopy(tf[:], ps_wx[3][:])
                for s in range(4):
                    pt = pst.tile([128, 128], F32, tag="pt")
                    nc.tensor.transpose(pt[:], tf[:, s * 128:(s + 1) * 128], ident[:])
                    tnm = wp.tile([128, 128], F32, tag="wxfnm")
                    cpcopy(tnm[:], pt[:])
                    r0 = ci * 512 + s * 128
                    nc.sync.dma_start(wxf_store[r0:r0 + 128, :], tnm[:])

            # ---------------- level phase ----------------
            if mode == "wx":
                levels_enabled = False
            else:
                levels_enabled = True
            lev_batches = {}
            for bi, b in enumerate(binfos):
                lev_batches.setdefault(b["L"], []).append(bi)

            for L in (range(nlev) if levels_enabled else []):
                bis = lev_batches[L]
                # sub-pass 1: gather + gates + h/c stores
                for bi in bis:
                    b = binfos[bi]
                    gs, bm, nch = b["gs"], b["bm"], b["nch"]
                    if L > 0:
                        co = b["icol_child"]
                        wo = b["icol_wxf"]
                        ic = nch * 8
                        gi = gtp.tile([128, MAXNCH, 384], F32, tag="gi")
                        nc.gpsimd.dma_gather(
                            out_ap=gi[:, :nch, :], in_ap=g_store[:, 0:384],
                            idxs_ap=idx_sb[:, co:co + ic],
                            num_idxs=nch * 128, num_idxs_reg=nch * 128,
                            elem_size=384, elem_step=512)
                        gh = gtp.tile([128, MAXNCH, 128], F32, tag="gh")
                        nc.gpsimd.dma_gather(
                            out_ap=gh[:, :nch, :], in_ap=g_store[:, 384:512],
                            idxs_ap=idx_sb[:, co:co + ic],
                            num_idxs=nch * 128, num_idxs_reg=nch * 128,
                            elem_size=128, elem_step=512)
                        gc = gtp.tile([128, MAXNCH, 128], F32, tag="gc")
                        nc.gpsimd.dma_gather(
                            out_ap=gc[:, :nch, :], in_ap=c_out[:, :],
                            idxs_ap=idx_sb[:, co:co + ic],
                            num_idxs=nch * 128, num_idxs_reg=nch * 128,
                            elem_size=128)
                        gw = gtp.tile([128, MAXNCH, 128], F32, tag="gw")
                        nc.gpsimd.dma_gather(
                            out_ap=gw[:, :nch, :], in_ap=wxf_store[:, :],
                            idxs_ap=idx_sb[:, wo:wo + ic],
                            num_idxs=nch * 128, num_idxs_reg=nch * 128,
                            elem_size=128)
                        ps_i = psA.tile([128, bm], F32, tag="A0")
                        ps_o = psA.tile([128, bm], F32, tag="A1")
                        ps_u = psA.tile([128, bm], F32, tag="A2")
                        ps_f = psA.tile([128, bm], F32, tag="A3")
                        for cidx, (wlo, win, so) in enumerate(b["chunks"]):
                            sAP = sall_sb[:, b["scol"] + so: b["scol"] + so + win]
                            t1 = wp.tile([128, 128], F32, tag="fc1")
                            nc.vector.tensor_add(t1[:], gh[:, cidx, :], gw[:, cidx, :])
                            t2 = wp.tile([128, 128], F32, tag="fc2")
                            nc.scalar.activation(t2[:], t1[:], SIG)
                            t3 = wp.tile([128, 128], F32, tag="fc3")
                            nc.vector.tensor_mul(t3[:], t2[:], gc[:, cidx, :])
                            nc.tensor.matmul(ps_f[:, wlo:wlo + win], t3[:], sAP,
                                             start=True, stop=True)
                            nc.tensor.matmul(ps_i[:, wlo:wlo + win],
                                             gi[:, cidx, 0:128], sAP,
                                             start=True, stop=True)
                            nc.tensor.matmul(ps_o[:, wlo:wlo + win],
                                             gi[:, cidx, 128:256], sAP,
                                             start=True, stop=True)
                            nc.tensor.matmul(ps_u[:, wlo:wlo + win],
                                             gi[:, cidx, 256:384], sAP,
                                             start=True, stop=True)
                    # load Wx slices for this batch
                    wxi = wp.tile([128, bm], F32, tag="wxi")
                    nc.sync.dma_start(wxi[:], wx_dram[:, 0 * N + gs: 0 * N + gs + bm])
                    wxo = wp.tile([128, bm], F32, tag="wxo")
                    nc.sync.dma_start(wxo[:], wx_dram[:, 1 * N + gs: 1 * N + gs + bm])
                    wxu = wp.tile([128, bm], F32, tag="wxu")
                    nc.sync.dma_start(wxu[:], wx_dram[:, 2 * N + gs: 2 * N + gs + bm])
                    i_sb = wp.tile([128, bm], F32, tag="isb")
                    o_sb = wp.tile([128, bm], F32, tag="osb")
                    u_sb = wp.tile([128, bm], F32, tag="usb")
                    c_sb = wp.tile([128, bm], F32, tag="csb")
                    h_sb = wp.tile([128, bm], F32, tag="hsb")
                    if L == 0:
                        nc.scalar.activation(i_sb[:], wxi[:], SIG)
                        nc.scalar.activation(o_sb[:], wxo[:], SIG)
                        nc.scalar.activation(u_sb[:], wxu[:], TANH)
                        nc.vector.tensor_mul(c_sb[:], i_sb[:], u_sb[:])
                    else:
                        t = wp.tile([128, bm], F32, tag="gtmp1")
                        nc.vector.tensor_add(t[:], ps_i[:], wxi[:])
                        nc.scalar.activation(i_sb[:], t[:], SIG)
                        t = wp.tile([128, bm], F32, tag="gtmp2")
                        nc.vector.tensor_add(t[:], ps_o[:], wxo[:])
                        nc.scalar.activation(o_sb[:], t[:], SIG)
                        t = wp.tile([128, bm], F32, tag="gtmp3")
                        nc.vector.tensor_add(t[:], ps_u[:], wxu[:])
                        nc.scalar.activation(u_sb[:], t[:], TANH)
                        t = wp.tile([128, bm], F32, tag="gtmp4")
                        nc.vector.tensor_mul(t[:], i_sb[:], u_sb[:])
                        nc.vector.tensor_add(c_sb[:], t[:], ps_f[:])
                    th = wp.tile([128, bm], F32, tag="thsb")
                    nc.scalar.activation(th[:], c_sb[:], TANH)
                    nc.vector.tensor_mul(h_sb[:], o_sb[:], th[:])
                    if ag_ins[bi] is not None:
                        h_r = wp.tile([128, bm], F32R, tag="hr")
                        nc.scalar.copy(h_r[:], h_sb[:])
                        nc.sync.dma_start(ag_ins[bi][:], h_r[:])
                    # h output stays feat-major (host untransposes);
                    # c needs node-major rows for the child gathers.
                    nc.sync.dma_start(h_out[:, gs:gs + bm], h_sb[:])
                    for s in range((bm + 127) // 128):
                        sw = min(128, bm - s * 128)
                        pt = pst.tile([128, 128], F32, tag="pt")
                        nc.tensor.transpose(
                            pt[:sw, :], c_sb[:, s * 128:s * 128 + sw], ident[:])
                        tnm = wp.tile([128, 128], F32, tag="cnm")
                        cpcopy(tnm[:sw, :], pt[:sw, :])
                        r0 = gs + s * 128
                        nc.sync.dma_start(c_out[r0:r0 + sw, :], tnm[:sw, :])

                # sub-pass 2: AG + g matmul + g store
                for bi in bis:
                    if mode == "noagg":
                        continue
                    b = binfos[bi]
                    if b["L"] == nlev - 1:
                        continue
                    gs, bm = b["gs"], b["bm"]
                    if mode == "nocc":
                        nc.sync.dma_start(ag_outs[bi][0:128, :], ag_ins[bi][:])
                    else:
                        nc.gpsimd.collective_compute(
                            "AllGather", mybir.AluOpType.bypass,
                            replica_groups=[list(range(NCORES))],
                            ins=[ag_ins[bi][:]], outs=[ag_outs[bi][:]])
                    hT = gtp.tile([128, KCH, bm], F32R, tag="hT", bufs=2)
                    nc.sync.dma_start(
                        hT[:], ag_outs[bi][:].rearrange("(k p) j -> p k j", p=128))
                    gbl = []
                    for blk in range(4):
                        psg = psA.tile([128, bm], F32, tag=f"A{blk}")
                        for k in range(KCH):
                            lhs = ut_sb[:, k, blk * 128:(blk + 1) * 128]
                            rhs = hT[:, k, :]
                            if bm < 256:
                                lhs = lhs.bitcast(F32)
                                rhs = rhs.bitcast(F32)
                            nc.tensor.matmul(psg[:], lhs, rhs,
                                             start=(k == 0), stop=(k == KCH - 1))
                        gs_sb = gtp.tile([128, bm], F32, tag=f"gsb{blk}", bufs=2)
                        cpcopy(gs_sb[:], psg[:])
                        gbl.append(gs_sb)
                    for s in range((bm + 127) // 128):
                        sw = min(128, bm - s * 128)
                        gnm = wp.tile([128, 512], F32, tag="gnm")
                        for blk in range(4):
                            pt = pst.tile([128, 128], F32, tag="pt")
                            nc.tensor.transpose(
                                pt[:sw, :], gbl[blk][:, s * 128:s * 128 + sw],
                                ident[:])
                            cpcopy(
                                gnm[:sw, blk * 128:(blk + 1) * 128], pt[:sw, :])
                        r0 = gs + s * 128
                        nc.sync.dma_start(g_store[r0:r0 + sw, :], gnm[:sw, :])

    nc.finalize()
    return nc


def kernel(x=None, head=None, **kw):
    import concourse.mybir as mybir  # noqa: F401  (env check)
    from concourse.bass_utils import run_bass_kernel_spmd

    x = np.asarray(x, np.float32)
    head_np = np.asarray(head)
    sched = _schedule(head_np)
    order = sched["order"]
    new_of_old = sched["new_of_old"]

    n = x.shape[0]
    # xT padded with bias row at row H (ones), zeros after; columns in new order
    xT = np.zeros((KCHX * 128, n), np.float32)
    xT[:H, :] = x[order].T
    xT[H, :] = 1.0

    Ws = {g: np.asarray(kw[f"W_{g}"], np.float32) for g in "iouf"}
    Us = {g: np.asarray(kw[f"U_{g}"], np.float32) for g in "iouf"}
    bs = {g: np.asarray(kw[f"b_{g}"], np.float32) for g in "iouf"}

    in_maps = []
    for c in range(NCORES):
        sl = slice(c * HC, (c + 1) * HC)
        WT = np.zeros((KCHX * 128, 512), np.float32)
        UT = np.zeros((H, 512), np.float32)
        for gi_, g in enumerate("iouf"):
            WT[:H, gi_ * 128:(gi_ + 1) * 128] = Ws[g][sl, :].T
            WT[H, gi_ * 128:(gi_ + 1) * 128] = bs[g][sl]
            UT[:, gi_ * 128:(gi_ + 1) * 128] = Us[g][sl, :].T
        in_maps.append({
            "xT": xT, "WT": WT, "UT": UT,
            "SALL": np.ascontiguousarray(sched["sall"]),
            "IDXT": np.ascontiguousarray(sched["idxt"]),
        })

    nc = _build_nc(sched)
    res = run_bass_kernel_spmd(nc, in_maps, list(range(NCORES)))

    h_new = np.concatenate([res.results[c]["h_out"] for c in range(NCORES)], axis=0).T
    c_new = np.concatenate([res.results[c]["c_out"][:n] for c in range(NCORES)], axis=1)
    h = h_new[new_of_old]
    cc = c_new[new_of_old]
    return h, cc
